# revision 9
# baseline (speedup 1.0000x reference)
"""CosFormer causal attention — Trainium2 Bass kernel, 8 NeuronCores.

Sharding: core i = (batch b = i//4, head-group g = i%4 covering heads 2g, 2g+1).
Each core computes the qkv projection for its two heads, chunked causal linear
attention (cos/sin feature channels, 128-wide chunks with a carried [2d, d+1]
state), and a partial output projection over its 128 context channels.
The host unshards by summing the 4 per-core partials of each batch (the output
projection's contraction is sharded over heads) and adding b_out.

Per-head q/k features live in [feat, t] layout as one [128, T] tile
(rows 0:64 = relu(q)*cos_t, 64:128 = relu(q)*sin_t), produced by projecting
with duplicated weight columns (matmul cost scales with N, not M, so the
duplication is nearly free) and one elementwise multiply with a stacked
[cos; sin] table.

Fully self-contained: hardcodes B=2, T=1024, E=512, H=8.
"""

import math
from contextlib import ExitStack

import numpy as np

import concourse.bass as bass
import concourse.mybir as mybir
import concourse.tile as tile
from concourse.bass_utils import run_bass_kernel_spmd
from concourse.masks import make_identity, make_upper_triangular
from concourse.vector_clock import ScopedClock

B, T, E = 2, 1024, 512
H, D = 8, 64
S = 128          # chunk size
NCHUNK = T // S  # 8
F32 = mybir.dt.float32
EPS = 1e-6


def _install_drain_patch():
    """This walrus build rejects a Drain carrying >1 sem wait. Split the
    Tile-exit drain's waits across single-wait SP nops."""
    if getattr(tile.TileContext, "_drain_patch_installed", False):
        return

    def _patched(self, tick_clock, wait_clock):
        nc = self.nc
        pre = nc.sync.nop(nofuse=True)
        wait_clock.add_sem_waits(pre.ins, ScopedClock({None: tick_clock.global_clock}))
        waits = list(pre.ins.sync_info.on_wait or []) if pre.ins.sync_info else []
        if len(waits) > 1:
            pre.ins.sync_info.on_wait = waits[:1]
            for w in waits[1:]:
                n = nc.sync.nop(nofuse=True)
                if n.ins.sync_info is None:
                    n.ins.sync_info = mybir.SyncInfo(on_wait=[w], on_update=[])
                else:
                    n.ins.sync_info.on_wait = [w]
        nc.sync.drain()
        nc.all_engine_barrier()
        popped = nc._tile_sem_poison_stack.pop()
        assert popped is self._sem_poison
        nc.clear_and_free_semaphores(list(self.sems.allocated().values()))
        nc.all_engine_barrier()

    tile.TileContext._drain_and_barrier = _patched
    tile.TileContext._drain_patch_installed = True


def _split_multi_waits(nc):
    """This walrus build only codegens ONE sync-wait command per instruction.
    Move excess waits onto same-engine NoOps inserted just before."""
    ctr = [0]

    def _mk_nop(engine, wait):
        ctr[0] += 1
        return mybir.InstNoOp(
            name=f"I-waitnop{ctr[0]}",
            engine=engine,
            ins=[],
            outs=[],
            sync_info=mybir.SyncInfo(on_wait=[wait], on_update=[]),
        )

    for f in nc.m.functions:
        for bb in f.blocks:
            new_insts = []
            for inst in bb.instructions:
                si = inst.sync_info
                waits = list(si.on_wait) if si and si.on_wait else []
                if len(waits) > 1:
                    for w in waits[:-1]:
                        new_insts.append(_mk_nop(inst.engine, w))
                    si.on_wait = waits[-1:]
                new_insts.append(inst)
            bb.instructions[:] = new_insts


def build_program() -> bass.Bass:
    _install_drain_patch()
    nc = bass.Bass()

    # wqkf: duplicated weight cols [qf_h0 | qf_h1 | kf_h0 | kf_h1], each 128 wide
    xt = nc.declare_dram_parameter("xt", [E, T], F32, isOutput=False)        # x[b].T
    wqkf = nc.declare_dram_parameter("wqkf", [E, 512], F32, isOutput=False)
    wvt = nc.declare_dram_parameter("wvt", [E, 128], F32, isOutput=False)    # [v0 v1].T
    bqkf = nc.declare_dram_parameter("bqkf", [512], F32, isOutput=False)     # dup'd biases
    bvrep = nc.declare_dram_parameter("bvrep", [128, 128], F32, isOutput=False)
    csrep = nc.declare_dram_parameter("csrep", [128, T], F32, isOutput=False)  # [cos;sin]
    w2 = nc.declare_dram_parameter("w2", [128, E], F32, isOutput=False)
    out = nc.declare_dram_parameter("out", [T, E], F32, isOutput=True)

    with tile.TileContext(nc) as tc, ExitStack() as ctx:
        singles = ctx.enter_context(tc.tile_pool(name="singles", bufs=1))
        vp_pool = ctx.enter_context(tc.tile_pool(name="vp", bufs=4))
        kf_pool = ctx.enter_context(tc.tile_pool(name="kf", bufs=4))
        atm_pool = ctx.enter_context(tc.tile_pool(name="atm", bufs=3))
        osb_pool = ctx.enter_context(tc.tile_pool(name="osb", bufs=2))
        nrm_pool = ctx.enter_context(tc.tile_pool(name="nrm", bufs=4))
        pp_big = ctx.enter_context(tc.tile_pool(name="pp_big", bufs=2, space="PSUM"))
        pp_mm = ctx.enter_context(tc.tile_pool(name="pp_mm", bufs=3, space="PSUM"))
        pp_kt = ctx.enter_context(tc.tile_pool(name="pp_kt", bufs=1, space="PSUM"))
        pp_cs = ctx.enter_context(tc.tile_pool(name="pp_cs", bufs=2, space="PSUM"))

        # ---- constant / input tiles -------------------------------------
        xt_s = singles.tile([128, 4, T], F32)
        nc.sync.dma_start(out=xt_s, in_=xt.rearrange("(kk p) t -> p kk t", p=128))
        wqkf_s = singles.tile([128, 4, 512], F32)
        nc.sync.dma_start(out=wqkf_s, in_=wqkf.rearrange("(kk p) c -> p kk c", p=128))
        wvt_s = singles.tile([128, 4, 128], F32)
        nc.sync.dma_start(out=wvt_s, in_=wvt.rearrange("(kk p) c -> p kk c", p=128))
        w2h = []
        for h in range(2):
            t_ = singles.tile([D, E], F32, name=f"w2h{h}")
            nc.sync.dma_start(out=t_, in_=w2[h * D:(h + 1) * D, :])
            w2h.append(t_)
        cs_s = singles.tile([128, T], F32)
        nc.sync.dma_start(out=cs_s, in_=csrep[:, :])
        bvrep_s = singles.tile([128, 128], F32)
        nc.sync.dma_start(out=bvrep_s, in_=bvrep[:, :])
        biases = []
        for bi in range(4):
            t_ = singles.tile([128, 1], F32, name=f"bias{bi}")
            nc.sync.dma_start(out=t_, in_=bqkf[bi * 128:(bi + 1) * 128, None])
            biases.append(t_)

        ident = singles.tile([128, 128], F32)
        make_identity(nc, ident)
        maskT = singles.tile([S, S], F32)
        make_upper_triangular(nc, maskT, val=1.0, diag=True)
        ones65 = singles.tile([65, D], F32)
        nc.vector.memset(ones65, 1.0)

        # per-head stacked feature tiles [cos;sin] x t
        qfT = [singles.tile([128, T], F32, name=f"qfT{h}") for h in range(2)]
        kfT = [singles.tile([128, T], F32, name=f"kfT{h}") for h in range(2)]
        ctxTh = [singles.tile([D, T], F32, name=f"ctxTh{h}") for h in range(2)]
        state = [singles.tile([128, D + 1], F32, name=f"state{h}") for h in range(2)]

        # ---- stage B+C: q/k features in [feat, t] layout ----------------
        # block bi: 0=qf_h0, 1=qf_h1, 2=kf_h0, 3=kf_h1
        for bi, dst in ((0, qfT[0]), (1, qfT[1]), (2, kfT[0]), (3, kfT[1])):
            for th in range(2):
                tslh = slice(th * 512, (th + 1) * 512)
                ps = pp_big.tile([128, 512], F32, tag="big", name=f"psB{bi}_{th}")
                for kk in range(4):
                    nc.tensor.matmul(
                        ps,
                        wqkf_s[:, kk, bi * 128:(bi + 1) * 128],
                        xt_s[:, kk, tslh],
                        start=(kk == 0),
                        stop=(kk == 3),
                    )
                # relu(Wx + b) on ACT, then *[cos;sin] on DVE
                nc.scalar.activation(
                    out=dst[:, tslh],
                    in_=ps,
                    func=mybir.ActivationFunctionType.Relu,
                    bias=biases[bi],
                    scale=1.0,
                )
                nc.vector.tensor_mul(dst[:, tslh], dst[:, tslh], cs_s[:, tslh])

        # ---- per chunk ---------------------------------------------------
        for tc_i in range(NCHUNK):
            tsl = slice(tc_i * S, (tc_i + 1) * S)

            # v projection for this chunk, [t, col] layout
            ps_v = pp_mm.tile([128, 128], F32, tag="mm", name=f"psv{tc_i}")
            for kk in range(4):
                nc.tensor.matmul(
                    ps_v,
                    xt_s[:, kk, tsl],
                    wvt_s[:, kk, :],
                    start=(kk == 0),
                    stop=(kk == 3),
                )
            vp = [vp_pool.tile([S, D + 1], F32, tag=f"vp{h}", name=f"vp{h}_{tc_i}")
                  for h in range(2)]
            for h in range(2):
                nc.vector.tensor_add(
                    vp[h][:, 0:D], ps_v[:, h * D:(h + 1) * D], bvrep_s[:, h * D:(h + 1) * D]
                )
                nc.gpsimd.memset(vp[h][:, D:D + 1], 1.0)

            # K_feat in [s, f] layout via PE transpose of kfT
            ps_kt = pp_kt.tile([128, 256], F32, tag="kt", name=f"pskt{tc_i}")
            kfeat = [kf_pool.tile([S, 128], F32, tag=f"kf{h}", name=f"kfeat{h}_{tc_i}")
                     for h in range(2)]
            for h in range(2):
                nc.tensor.transpose(ps_kt[:, h * 128:(h + 1) * 128], kfT[h][:, tsl], ident)
                nc.vector.tensor_copy(kfeat[h], ps_kt[:, h * 128:(h + 1) * 128])

            for h in range(2):
                # A^T = Kf^T Qf (within chunk), [s, t]
                ps_a = pp_mm.tile([128, 128], F32, tag="mm", name=f"psa{h}_{tc_i}")
                nc.tensor.matmul(ps_a, kfT[h][:, tsl], qfT[h][:, tsl], start=True, stop=True)
                atm = atm_pool.tile([S, S], F32, tag="atm", name=f"atm{h}_{tc_i}")
                nc.vector.tensor_mul(atm, ps_a, maskT)

                # ctx^T (+norm row 64) = V'^T A^T_masked + S'^T Qf
                ps_c = pp_cs.tile([D + 1, S], F32, tag="cs", name=f"psc{h}_{tc_i}")
                if tc_i > 0:
                    nc.tensor.matmul(ps_c, state[h], qfT[h][:, tsl], start=True, stop=False)
                    nc.tensor.matmul(ps_c, vp[h], atm, start=False, stop=True)
                else:
                    nc.tensor.matmul(ps_c, vp[h], atm, start=True, stop=True)

                # state += Kf^T V'
                ps_s = pp_cs.tile([128, D + 1], F32, tag="cs", name=f"pss{h}_{tc_i}")
                nc.tensor.matmul(ps_s, kfeat[h], vp[h], start=True, stop=True)
                if tc_i == 0:
                    nc.vector.tensor_copy(state[h], ps_s)
                else:
                    nc.vector.tensor_add(state[h], state[h], ps_s)

                # normalize: ctxT_h = ctx_unnorm * 1/(norm + eps), bcast via PE
                nrm = nrm_pool.tile([D + 1, S], F32, tag="nrm", name=f"nrm{h}_{tc_i}")
                nc.vector.tensor_scalar_add(nrm[D:D + 1, :], ps_c[D:D + 1, :], EPS)
                nc.vector.reciprocal(nrm[D:D + 1, :], nrm[D:D + 1, :])
                ps_b = pp_mm.tile([D, S], F32, tag="mm", name=f"psb{h}_{tc_i}")
                nc.tensor.matmul(ps_b, ones65[D:D + 1, :], nrm[D:D + 1, :],
                                 start=True, stop=True)
                ctxu = nrm_pool.tile([D, S], F32, tag="ctxu", name=f"ctxu{h}_{tc_i}")
                nc.scalar.copy(ctxu, ps_c[0:D, :])
                nc.vector.tensor_mul(ctxTh[h][:, tsl], ctxu, ps_b)

            # partial out-projection for this chunk: [t, E]
            ps_o = pp_big.tile([128, E], F32, tag="big", name=f"pso{tc_i}")
            nc.tensor.matmul(ps_o, ctxTh[0][:, tsl], w2h[0], start=True, stop=False)
            nc.tensor.matmul(ps_o, ctxTh[1][:, tsl], w2h[1], start=False, stop=True)
            o_s = osb_pool.tile([128, E], F32, tag="osb", name=f"os{tc_i}")
            nc.scalar.copy(o_s, ps_o)
            nc.sync.dma_start(out=out[tsl, :], in_=o_s)

    _split_multi_waits(nc)
    return nc


_PROGRAM = None


def _get_program():
    global _PROGRAM
    if _PROGRAM is None:
        _PROGRAM = build_program()
    return _PROGRAM


def _make_in_maps(x, w_qkv, b_qkv, w_out):
    pos = np.arange(T, dtype=np.float32)
    ang = (math.pi / 2) * pos / T
    cosw = np.cos(ang).astype(np.float32)
    sinw = np.sin(ang).astype(np.float32)
    csrep = np.concatenate([
        np.broadcast_to(cosw[None, :], (D, T)),
        np.broadcast_to(sinw[None, :], (D, T)),
    ], 0).astype(np.float32)

    in_maps = []
    for i in range(8):
        b, g = divmod(i, 4)
        h0, h1 = 2 * g, 2 * g + 1
        wq = lambda h: w_qkv[h * D:(h + 1) * D]
        wk = lambda h: w_qkv[E + h * D:E + (h + 1) * D]
        wv = lambda h: w_qkv[2 * E + h * D:2 * E + (h + 1) * D]
        bq = lambda h: b_qkv[h * D:(h + 1) * D]
        bk = lambda h: b_qkv[E + h * D:E + (h + 1) * D]
        bv = lambda h: b_qkv[2 * E + h * D:2 * E + (h + 1) * D]
        hcols = np.r_[h0 * D:(h0 + 1) * D, h1 * D:(h1 + 1) * D]
        # duplicated blocks: [qf_h0 | qf_h1 | kf_h0 | kf_h1], each [128, E] -> .T
        wqkf = np.concatenate([
            wq(h0), wq(h0), wq(h1), wq(h1), wk(h0), wk(h0), wk(h1), wk(h1)
        ], 0).T
        bqkf = np.concatenate([
            bq(h0), bq(h0), bq(h1), bq(h1), bk(h0), bk(h0), bk(h1), bk(h1)
        ])
        in_maps.append({
            "xt": np.ascontiguousarray(x[b].T),
            "wqkf": np.ascontiguousarray(wqkf),
            "wvt": np.ascontiguousarray(np.concatenate([wv(h0), wv(h1)], 0).T),
            "bqkf": np.ascontiguousarray(bqkf),
            "bvrep": np.broadcast_to(
                np.concatenate([bv(h0), bv(h1)])[None, :], (128, 128)).copy(),
            "csrep": csrep,
            "w2": np.ascontiguousarray(w_out[:, hcols].T),
        })
    return in_maps


def run(inputs, trace=False):
    x = np.asarray(inputs["x"], dtype=np.float32)
    w_qkv = np.asarray(inputs["w_qkv"], dtype=np.float32)
    b_qkv = np.asarray(inputs["b_qkv"], dtype=np.float32)
    w_out = np.asarray(inputs["w_out"], dtype=np.float32)
    b_out = np.asarray(inputs["b_out"], dtype=np.float32)

    nc = _get_program()
    in_maps = _make_in_maps(x, w_qkv, b_qkv, w_out)
    res = run_bass_kernel_spmd(nc, in_maps, list(range(8)), trace=trace)

    out = np.empty((B, T, E), dtype=np.float32)
    for b in range(B):
        acc = res.results[4 * b]["out"].astype(np.float32)
        for g in range(1, 4):
            acc = acc + res.results[4 * b + g]["out"]
        out[b] = acc + b_out[None, :]
    return out, res


def kernel(**inputs) -> np.ndarray:
    out, _ = run(inputs, trace=False)
    return out


# revision 24
# speedup vs baseline: 2.0906x; 2.0906x over previous
"""CosFormer causal attention — Trainium2 Bass kernel, 8 NeuronCores.

Sharding: core i = (batch b = i//4, head-group g = i%4 covering heads 2g, 2g+1).
Each core computes the qkv projection for its two heads, chunked causal linear
attention (cos/sin feature channels, 128-wide chunks with a carried [2d, d+1]
state), and a partial output projection over its 128 context channels.
The host unshards by summing the 4 per-core partials of each batch (the output
projection's contraction is sharded over heads) and adding b_out.

Per-head q/k features live in [feat, t] layout as one [128, T] tile
(rows 0:64 = relu(q)*cos_t, 64:128 = relu(q)*sin_t), produced by projecting
with duplicated weight columns (matmul cost scales with N, not M, so the
duplication is nearly free) and one elementwise multiply with a stacked
[cos; sin] table.

Fully self-contained: hardcodes B=2, T=1024, E=512, H=8.
"""

import math
from contextlib import ExitStack

import numpy as np

import concourse.bass as bass
import concourse.mybir as mybir
import concourse.tile as tile
from concourse.bass_utils import run_bass_kernel_spmd
from concourse.masks import make_identity, make_upper_triangular
from concourse.vector_clock import ScopedClock

B, T, E = 2, 1024, 512
H, D = 8, 64
S = 128          # chunk size
NCHUNK = T // S  # 8
F32 = mybir.dt.float32
F32R = mybir.dt.float32r
EPS = 1e-6


def _install_drain_patch():
    """This walrus build rejects a Drain carrying >1 sem wait. Split the
    Tile-exit drain's waits across single-wait SP nops."""
    if getattr(tile.TileContext, "_drain_patch_installed", False):
        return

    def _patched(self, tick_clock, wait_clock):
        nc = self.nc
        pre = nc.sync.nop(nofuse=True)
        wait_clock.add_sem_waits(pre.ins, ScopedClock({None: tick_clock.global_clock}))
        waits = list(pre.ins.sync_info.on_wait or []) if pre.ins.sync_info else []
        if len(waits) > 1:
            pre.ins.sync_info.on_wait = waits[:1]
            for w in waits[1:]:
                n = nc.sync.nop(nofuse=True)
                if n.ins.sync_info is None:
                    n.ins.sync_info = mybir.SyncInfo(on_wait=[w], on_update=[])
                else:
                    n.ins.sync_info.on_wait = [w]
        nc.sync.drain()
        nc.all_engine_barrier()
        popped = nc._tile_sem_poison_stack.pop()
        assert popped is self._sem_poison
        nc.clear_and_free_semaphores(list(self.sems.allocated().values()))
        nc.all_engine_barrier()

    tile.TileContext._drain_and_barrier = _patched
    tile.TileContext._drain_patch_installed = True


def _split_multi_waits(nc):
    """This walrus build only codegens ONE sync-wait command per instruction.
    Move excess waits onto same-engine NoOps inserted just before."""
    ctr = [0]

    def _mk_nop(engine, wait):
        ctr[0] += 1
        return mybir.InstNoOp(
            name=f"I-waitnop{ctr[0]}",
            engine=engine,
            ins=[],
            outs=[],
            sync_info=mybir.SyncInfo(on_wait=[wait], on_update=[]),
        )

    for f in nc.m.functions:
        for bb in f.blocks:
            new_insts = []
            for inst in bb.instructions:
                si = inst.sync_info
                waits = list(si.on_wait) if si and si.on_wait else []
                if len(waits) > 1:
                    for w in waits[:-1]:
                        new_insts.append(_mk_nop(inst.engine, w))
                    si.on_wait = waits[-1:]
                new_insts.append(inst)
            bb.instructions[:] = new_insts


def build_program() -> bass.Bass:
    _install_drain_patch()
    nc = bass.Bass()

    # wqkf: duplicated weight cols [qf_h0 | qf_h1 | kf_h0 | kf_h1], each 128 wide
    xt = nc.declare_dram_parameter("xt", [E, T], F32R, isOutput=False)        # x[b].T
    wqkf = nc.declare_dram_parameter("wqkf", [E, 512], F32R, isOutput=False)
    wvt = nc.declare_dram_parameter("wvt", [E, 128], F32R, isOutput=False)    # [v0 v1].T
    bqkf = nc.declare_dram_parameter("bqkf", [640], F32, isOutput=False)     # dup'd qk biases + v bias
    csrep = nc.declare_dram_parameter("csrep", [128, T], F32, isOutput=False)  # [cos;sin]
    w2 = nc.declare_dram_parameter("w2", [128, E], F32R, isOutput=False)
    identin = nc.declare_dram_parameter("identin", [128, 128], F32R, isOutput=False)
    out = nc.declare_dram_parameter("out", [T, E], F32, isOutput=True)

    with tile.TileContext(nc) as tc, ExitStack() as ctx:
        singles = ctx.enter_context(tc.tile_pool(name="singles", bufs=1))
        vp_pool = ctx.enter_context(tc.tile_pool(name="vp", bufs=4))
        kf_pool = ctx.enter_context(tc.tile_pool(name="kf", bufs=4))
        atm_pool = ctx.enter_context(tc.tile_pool(name="atm", bufs=3))
        osb_pool = ctx.enter_context(tc.tile_pool(name="osb", bufs=2))
        nrm_pool = ctx.enter_context(tc.tile_pool(name="nrm", bufs=4))
        pp_big = ctx.enter_context(tc.tile_pool(name="pp_big", bufs=2, space="PSUM"))
        pp_mm = ctx.enter_context(tc.tile_pool(name="pp_mm", bufs=2, space="PSUM"))
        pp_psn = ctx.enter_context(tc.tile_pool(name="pp_psn", bufs=1, space="PSUM"))
        pp_kt = ctx.enter_context(tc.tile_pool(name="pp_kt", bufs=1, space="PSUM"))
        pp_cs = ctx.enter_context(tc.tile_pool(name="pp_cs", bufs=2, space="PSUM"))

        # ---- constant / input tiles -------------------------------------
        xt_s = singles.tile([128, 4, T], F32R)
        nc.sync.dma_start(out=xt_s, in_=xt.rearrange("(kk p) t -> p kk t", p=128))
        wqkf_s = singles.tile([128, 4, 512], F32R)
        nc.sync.dma_start(out=wqkf_s, in_=wqkf.rearrange("(kk p) c -> p kk c", p=128))
        wvt_s = singles.tile([128, 4, 128], F32R)
        nc.sync.dma_start(out=wvt_s, in_=wvt.rearrange("(kk p) c -> p kk c", p=128))
        w2h = []
        for h in range(2):
            t_ = singles.tile([D, E], F32R, name=f"w2h{h}")
            nc.sync.dma_start(out=t_, in_=w2[h * D:(h + 1) * D, :])
            w2h.append(t_)
        cs_s = singles.tile([128, T], F32)
        nc.sync.dma_start(out=cs_s, in_=csrep[:, :])
        biases = []
        for bi in range(4):
            t_ = singles.tile([128, 1], F32, name=f"bias{bi}")
            nc.sync.dma_start(out=t_, in_=bqkf[bi * 128:(bi + 1) * 128, None])
            biases.append(t_)
        bias_v = singles.tile([128, 1], F32, name="bias_v")
        nc.sync.dma_start(out=bias_v, in_=bqkf[512:640, None])

        ident = singles.tile([128, 128], F32R)
        nc.sync.dma_start(out=ident, in_=identin[:, :])
        maskT = singles.tile([S, S], F32)
        make_upper_triangular(nc, maskT, val=1.0, diag=True)
        eps_t = singles.tile([1, 1], F32, name="eps_t")
        nc.vector.memset(eps_t, EPS)
        onesz_col = singles.tile([128, 2], F32, name="onesz_col")
        nc.vector.memset(onesz_col[:, 0:1], 1.0)
        nc.vector.memset(onesz_col[:, 1:2], 0.0)

        # per-head stacked feature tiles [cos;sin] x t
        qfT = [singles.tile([128, T], F32R, name=f"qfT{h}") for h in range(2)]
        kfT = [singles.tile([128, T], F32R, name=f"kfT{h}") for h in range(2)]
        ctxTh = [singles.tile([D, T], F32, name=f"ctxTh{h}") for h in range(2)]
        state = [singles.tile([128, D + 2], F32R, name=f"state{h}") for h in range(2)]

        # ---- stage B+C: q/k features in [feat, t] layout ----------------
        # block bi: 0=qf_h0, 1=qf_h1, 2=kf_h0, 3=kf_h1
        for bi, dst in ((0, qfT[0]), (1, qfT[1]), (2, kfT[0]), (3, kfT[1])):
            for th in range(2):
                tslh = slice(th * 512, (th + 1) * 512)
                ps = pp_big.tile([128, 512], F32, tag="big", name=f"psB{bi}_{th}")
                for kk in range(4):
                    nc.tensor.matmul(
                        ps,
                        wqkf_s[:, kk, bi * 128:(bi + 1) * 128],
                        xt_s[:, kk, tslh],
                        start=(kk == 0),
                        stop=(kk == 3),
                    )
                # relu(Wx + b) on ACT, then *[cos;sin] on DVE
                nc.scalar.activation(
                    out=dst[:, tslh],
                    in_=ps,
                    func=mybir.ActivationFunctionType.Relu,
                    bias=biases[bi],
                    scale=1.0,
                )
                nc.vector.tensor_mul(dst[:, tslh], dst[:, tslh], cs_s[:, tslh])

        # ---- stage D: v^T projection, [col, t] layout (bias fused in ACT) --
        vT = singles.tile([128, T], F32R, name="vT")
        for th in range(2):
            tslh = slice(th * 512, (th + 1) * 512)
            ps = pp_big.tile([128, 512], F32, tag="big", name=f"psV{th}")
            for kk in range(4):
                nc.tensor.matmul(
                    ps,
                    wvt_s[:, kk, :],
                    xt_s[:, kk, tslh],
                    start=(kk == 0),
                    stop=(kk == 3),
                )
            nc.scalar.activation(
                out=vT[:, tslh],
                in_=ps,
                func=mybir.ActivationFunctionType.Identity,
                bias=bias_v,
                scale=1.0,
            )

        # ---- per chunk ---------------------------------------------------
        for tc_i in range(NCHUNK):
            tsl = slice(tc_i * S, (tc_i + 1) * S)

            # transpose kfT (both heads) and vT for this chunk: [*, t] -> [t, *]
            ps_kt = pp_kt.tile([128, 384], F32R, tag="kt", name=f"pskt{tc_i}")
            kfeat = [kf_pool.tile([S, 128], F32R, tag=f"kf{h}", name=f"kfeat{h}_{tc_i}")
                     for h in range(2)]
            vp = [vp_pool.tile([S, D + 2], F32R, tag=f"vp{h}", name=f"vp{h}_{tc_i}")
                  for h in range(2)]
            for h in range(2):
                nc.tensor.transpose(ps_kt[:, h * 128:(h + 1) * 128], kfT[h][:, tsl], ident)
                nc.scalar.copy(kfeat[h], ps_kt[:, h * 128:(h + 1) * 128])
            nc.tensor.transpose(ps_kt[:, 256:384], vT[:, tsl], ident)
            for h in range(2):
                nc.scalar.copy(vp[h][:, 0:D], ps_kt[:, 256 + h * D:256 + (h + 1) * D])
                nc.scalar.copy(vp[h][:, D:D + 2], onesz_col)

            ps_o = [None, None]
            nrm_col = [None, None]
            for h in range(2):
                # A^T = Kf^T Qf (within chunk), [s, t]
                ps_a = pp_mm.tile([128, 128], F32, tag="mm", name=f"psa{h}_{tc_i}")
                nc.tensor.matmul(ps_a, kfT[h][:, tsl], qfT[h][:, tsl], start=True, stop=True)
                atm = atm_pool.tile([S, S], F32R, tag="atm", name=f"atm{h}_{tc_i}")
                nc.vector.tensor_mul(atm, ps_a, maskT)

                # ctx^T (+norm row 64) = V'^T A^T_masked + S'^T Qf
                ps_c = pp_cs.tile([D + 2, S], F32, tag="cs", name=f"psc{h}_{tc_i}")
                if tc_i > 0:
                    nc.tensor.matmul(ps_c, state[h], qfT[h][:, tsl], start=True, stop=False)
                    nc.tensor.matmul(ps_c, vp[h], atm, start=False, stop=True)
                else:
                    nc.tensor.matmul(ps_c, vp[h], atm, start=True, stop=True)

                # state += Kf^T V'
                ps_s = pp_cs.tile([128, D + 2], F32, tag="cs", name=f"pss{h}_{tc_i}")
                nc.tensor.matmul(ps_s, kfeat[h], vp[h], start=True, stop=True)
                if tc_i == 0:
                    nc.vector.tensor_copy(state[h], ps_s)
                else:
                    nc.vector.tensor_add(state[h], state[h], ps_s)

                # norm+eps -> row [1,S] -> PE transpose -> column [S,1] -> recip
                nrow = nrm_pool.tile([1, S], F32R, tag="nrow", name=f"nrow{h}_{tc_i}")
                nc.scalar.activation(out=nrow, in_=ps_c[D:D + 1, :],
                                     func=mybir.ActivationFunctionType.Identity,
                                     bias=eps_t[0:1, 0:1], scale=1.0)
                ps_n = pp_psn.tile([S, 2], F32R, tag="psn", name=f"psn{h}_{tc_i}")
                nc.tensor.transpose(ps_n, nrow, ident[0:1, 0:2])
                ncol = nrm_pool.tile([S, 1], F32, tag="ncol", name=f"ncol{h}_{tc_i}")
                nc.vector.reciprocal(ncol, ps_n[:, 0:1])
                nrm_col[h] = ncol

                # unnormalized ctx -> SBUF, per-head partial out-projection
                ctxu = nrm_pool.tile([D, S], F32R, tag="ctxu", name=f"ctxu{h}_{tc_i}")
                nc.scalar.copy(ctxu, ps_c[0:D, :])
                ps_oh = pp_big.tile([128, E], F32, tag="big", name=f"pso{h}_{tc_i}")
                nc.tensor.matmul(ps_oh, ctxu, w2h[h], start=True, stop=True)
                ps_o[h] = ps_oh

            # scale each head's projection by its 1/norm (per-partition) and sum
            o_s = osb_pool.tile([128, E], F32, tag="osb", name=f"os{tc_i}")
            nc.vector.tensor_scalar_mul(o_s, ps_o[0], nrm_col[0])
            nc.vector.scalar_tensor_tensor(
                out=o_s, in0=ps_o[1], scalar=nrm_col[1], in1=o_s,
                op0=mybir.AluOpType.mult, op1=mybir.AluOpType.add,
            )
            nc.sync.dma_start(out=out[tsl, :], in_=o_s)

    _split_multi_waits(nc)
    return nc


_PROGRAM = None


def _get_program():
    global _PROGRAM
    if _PROGRAM is None:
        _PROGRAM = build_program()
    return _PROGRAM


def _make_in_maps(x, w_qkv, b_qkv, w_out):
    pos = np.arange(T, dtype=np.float32)
    ang = (math.pi / 2) * pos / T
    cosw = np.cos(ang).astype(np.float32)
    sinw = np.sin(ang).astype(np.float32)
    csrep = np.concatenate([
        np.broadcast_to(cosw[None, :], (D, T)),
        np.broadcast_to(sinw[None, :], (D, T)),
    ], 0).astype(np.float32)

    in_maps = []
    for i in range(8):
        b, g = divmod(i, 4)
        h0, h1 = 2 * g, 2 * g + 1
        wq = lambda h: w_qkv[h * D:(h + 1) * D]
        wk = lambda h: w_qkv[E + h * D:E + (h + 1) * D]
        wv = lambda h: w_qkv[2 * E + h * D:2 * E + (h + 1) * D]
        bq = lambda h: b_qkv[h * D:(h + 1) * D]
        bk = lambda h: b_qkv[E + h * D:E + (h + 1) * D]
        bv = lambda h: b_qkv[2 * E + h * D:2 * E + (h + 1) * D]
        hcols = np.r_[h0 * D:(h0 + 1) * D, h1 * D:(h1 + 1) * D]
        # duplicated blocks: [qf_h0 | qf_h1 | kf_h0 | kf_h1], each [128, E] -> .T
        wqkf = np.concatenate([
            wq(h0), wq(h0), wq(h1), wq(h1), wk(h0), wk(h0), wk(h1), wk(h1)
        ], 0).T
        bqkf = np.concatenate([
            bq(h0), bq(h0), bq(h1), bq(h1), bk(h0), bk(h0), bk(h1), bk(h1),
            bv(h0), bv(h1)
        ])
        in_maps.append({
            "xt": np.ascontiguousarray(x[b].T),
            "wqkf": np.ascontiguousarray(wqkf),
            "wvt": np.ascontiguousarray(np.concatenate([wv(h0), wv(h1)], 0).T),
            "bqkf": np.ascontiguousarray(bqkf),
            "csrep": csrep,
            "w2": np.ascontiguousarray(w_out[:, hcols].T),
            "identin": np.eye(128, dtype=np.float32),
        })
    return in_maps


def run(inputs, trace=False):
    x = np.asarray(inputs["x"], dtype=np.float32)
    w_qkv = np.asarray(inputs["w_qkv"], dtype=np.float32)
    b_qkv = np.asarray(inputs["b_qkv"], dtype=np.float32)
    w_out = np.asarray(inputs["w_out"], dtype=np.float32)
    b_out = np.asarray(inputs["b_out"], dtype=np.float32)

    nc = _get_program()
    in_maps = _make_in_maps(x, w_qkv, b_qkv, w_out)
    res = run_bass_kernel_spmd(nc, in_maps, list(range(8)), trace=trace)

    out = np.empty((B, T, E), dtype=np.float32)
    for b in range(B):
        acc = res.results[4 * b]["out"].astype(np.float32)
        for g in range(1, 4):
            acc = acc + res.results[4 * b + g]["out"]
        out[b] = acc + b_out[None, :]
    return out, res


def kernel(**inputs) -> np.ndarray:
    out, _ = run(inputs, trace=False)
    return out
